# revision 28
# baseline (speedup 1.0000x reference)
"""MultiHeadDiffAttn Trainium2 kernel (v8).

Sharding: 8 cores = 4-way data parallel over batch x 2-way tensor parallel
over heads (8 v-heads / 16 half-heads per core).  Host sums the two
partial output projections per batch.

v8 = v6's fp16 matmul dataflow + an ACT-saturating schedule.  (fp8
DoubleRow was measured on hardware to stream at the same 1 column/cycle
as fp16 -- the cost model's 0.5 cyc/col is simulator-only -- so fp8
buys nothing here and v7's compensated-fp8 machinery was dropped.)

  - exp fused over both half-head e's: one ACT op per (head, key block)
    reading the [128, 2, T-t0] PSUM pair, 64 ops instead of 128, cutting
    ACT busy from ~102us to ~79us.  Causal masks fused the same way.
  - the qkv projection is interleaved into the head loop: matmul groups
    for later heads ride the U PSUM banks between epilogue reads and the
    next head's deferred AV block, so the first exp fires ~9us in (vs
    ~47us in v6) and ACT stays saturated.
  - per-head AV matmuls are deferred to the end of the head (es tiles
    live in a 9-deep pool), which keeps the PSUM tag rotation free of
    mid-head deadlocks with the qkv feeds.
  - qTp zero padding via DMA zero-fill (frees ~17us of DVE/GpSimd).
  - output-projection tail: per-tj transposes split in half across two
    DMA queues.
"""

import math
from contextlib import ExitStack

import numpy as np

import concourse.bass as bass
import concourse.tile as tile
from concourse import mybir
from concourse.bass_utils import run_bass_kernel_spmd

_MAX_WAITS = 1


def _legalize_sync_waits(d):
    for f in d.get("functions", []):
        for bb in f.get("blocks", []):
            out = []
            for inst in bb["instructions"]:
                si = inst.get("sync_info")
                waits = (si or {}).get("on_wait") or []
                if len(waits) > _MAX_WAITS:
                    extra = waits[: len(waits) - _MAX_WAITS]
                    keep = waits[len(waits) - _MAX_WAITS :]
                    for j in range(0, len(extra), _MAX_WAITS):
                        nop = {
                            "engine": inst["engine"],
                            "ins": [],
                            "outs": [],
                            "name": f"{inst['name']}-lw{j}",
                            "opcode": "NoOp",
                            "sync_info": {
                                "on_wait": extra[j : j + _MAX_WAITS],
                                "on_update": [],
                            },
                        }
                        if "debug" in inst:
                            nop["debug"] = inst["debug"]
                        out.append(nop)
                    si["on_wait"] = keep
                out.append(inst)
            bb["instructions"] = out
    return d


_orig_to_json_bytes = bass.Bass.to_json_bytes


def _patched_to_json_bytes(self, *a, **kw):
    import json as _json

    raw = _orig_to_json_bytes(self, *a, **kw)
    return _json.dumps(_legalize_sync_waits(_json.loads(raw))).encode()


bass.Bass.to_json_bytes = _patched_to_json_bytes

F32 = mybir.dt.float32
F16 = mybir.dt.float16

B, T, C = 4, 1024, 1024
H_TOT = 16
HD = 32
DV = 64
G = 2
HPG = H_TOT // G
LAMBDA_INIT = 0.8 - 0.6 * math.exp(-0.3 * (1 - 1))
EPS = 1e-5
N_CORES = 8

NT = T // 128
NKC = C // 128
ES_BUFS = 9


def _emit(ctx: ExitStack, tc: tile.TileContext, prm, y):
    nc = tc.nc
    AluOp = mybir.AluOpType
    Act = mybir.ActivationFunctionType

    const = ctx.enter_context(tc.tile_pool(name="const", bufs=1))
    lam_sb = const.tile([128, 1], F32)
    nc.sync.dma_start(out=lam_sb, in_=prm["lam"][:])
    eps_sb = const.tile([128, 1], F32)
    nc.vector.memset(eps_sb, EPS)

    big = ctx.enter_context(tc.tile_pool(name="big", bufs=1))
    xT_sb = big.tile([128, NKC, T], F16)
    wqk_sb = big.tile([128, NKC, 1024], F16)
    wv_sb = big.tile([128, NKC, 512], F16)
    wp_sb = big.tile([128, 4, C], F16)
    qkT_sb = big.tile([128, NKC, T], F16)
    qTp_sb = big.tile([128, 2 * HPG, T], F16)
    VW = 68  # v row stride: DV+1 used, padded for alignment
    v_sb = big.tile([128, NT, HPG, VW], F16)
    outcat_sb = big.tile([128, NT, HPG * DV], F16)
    outcatT_sb = big.tile([128, 4, T], F16)

    # es tiles for a whole head stay alive until its deferred AV block
    es_pool = ctx.enter_context(tc.tile_pool(name="es", bufs=ES_BUFS))

    # ---- input loads ----
    nc.sync.dma_start(
        out=xT_sb, in_=prm["xT"][:].rearrange("(c p) t -> p c t", p=128)
    )
    nc.scalar.dma_start(
        out=wqk_sb, in_=prm["w_qk"][:].rearrange("(c p) m -> p c m", p=128)
    )
    nc.gpsimd.dma_start(
        out=wv_sb, in_=prm["w_v"][:].rearrange("(c p) m -> p c m", p=128)
    )
    nc.gpsimd.dma_start(
        out=wp_sb, in_=prm["w_p"][:].rearrange("(c p) m -> p c m", p=128)
    )

    # zero padding of qTp via DMA fill, in row-band x half slabs so the
    # first head's scatters do not wait on one monolithic transfer
    zz = prm["zz"]
    for j in range(4):
        rows = slice(j * 32, (j + 1) * 32)
        for half in range(2):
            hs = slice(half * HPG, (half + 1) * HPG)
            eng = nc.sync if (j + half) % 2 == 0 else nc.gpsimd
            eng.dma_start(
                out=qTp_sb[rows, hs, :],
                in_=zz[rows, None, :].broadcast_to([32, HPG, T]),
            )

    # ---- qkv building blocks ----
    mm_tags = ["u00", "u01", "u10", "u11"]
    up_pool = [None]  # set once the psum pool opens

    def emit_qk_group(cc, nh, slot):
        """q or k feature block cc (0..3 q, 4..7 k), time half nh."""
        ps = up_pool[0].tile(
            [128, 512], F32, tag=mm_tags[slot % 4], name=f"qk{cc}{nh}"
        )
        for kc in range(NKC):
            nc.tensor.matmul(
                ps,
                wqk_sb[:, kc, cc * 128 : (cc + 1) * 128],
                xT_sb[:, kc, nh * 512 : (nh + 1) * 512],
                start=(kc == 0),
                stop=(kc == NKC - 1),
            )
        cols = slice(nh * 512, (nh + 1) * 512)
        nc.vector.tensor_copy(out=qkT_sb[:, cc, cols], in_=ps)
        if cc < 4:
            # scatter q into its 32-row band of qTp (gpsimd: SBUF->SBUF)
            for j in range(4):
                hh = (cc % 4) * 4 + j
                rows = slice(j * 32, (j + 1) * 32)
                nc.gpsimd.tensor_copy(
                    out=qTp_sb[rows, hh, cols], in_=qkT_sb[rows, cc, cols]
                )

    def emit_v_group(tt, slot):
        ps = up_pool[0].tile([128, 512], F32, tag=mm_tags[slot % 4], name=f"v{tt}")
        for kc in range(NKC):
            nc.tensor.matmul(
                ps,
                xT_sb[:, kc, tt * 128 : (tt + 1) * 128],
                wv_sb[:, kc, :],
                start=(kc == 0),
                stop=(kc == NKC - 1),
            )
        nc.vector.tensor_copy(
            out=v_sb[:, tt, :, 0:DV],
            in_=ps[:].rearrange("p (h d) -> p h d", h=HPG),
        )
        nc.vector.memset(v_sb[:, tt, :, DV : DV + 1], 1.0)

    with (
        tc.tile_pool(name="sps", bufs=1, space="PSUM") as s_pool,
        tc.tile_pool(name="ups", bufs=1, space="PSUM") as u_pool,
        tc.tile_pool(name="comb", bufs=3) as comb,
        tc.tile_pool(name="ohp", bufs=1, space="SBUF") as ohp,
    ):
        up_pool[0] = u_pool
        oh_sb = ohp.tile([128, HPG, NT, DV], F32, tag="ohall", name="ohall")
        ssq_all = ohp.tile([128, HPG * NT], F32, tag="ssqall", name="ssqall")

        # prologue: q/k for heads 0,1 plus the first v block
        emit_qk_group(0, 0, 0)
        emit_qk_group(4, 0, 1)
        emit_qk_group(0, 1, 2)
        emit_qk_group(4, 1, 3)
        emit_v_group(0, 0)

        # remaining qkv work doled out inside the head loop (all feeds of
        # head h are emitted before head h's u-bank allocation so the PSUM
        # tag rotation never waits on the in-flight head)
        feed = {
            (0, 1): [("v", 1)],
            (0, 2): [("v", 2)],
            (0, 3): [("v", 3)],
            (0, 4): [("v", 4)],
            (0, 5): [("v", 5)],
            (0, 6): [("v", 6)],
            (0, 7): [("v", 7)],
            (1, 3): [("qk", 1, 0)],
            (1, 4): [("qk", 1, 1)],
            (1, 5): [("qk", 5, 0)],
            (1, 6): [("qk", 5, 1)],
            (2, 3): [("qk", 2, 0)],
            (2, 4): [("qk", 2, 1)],
            (2, 5): [("qk", 6, 0)],
            (2, 6): [("qk", 6, 1)],
            (3, 3): [("qk", 3, 0)],
            (3, 4): [("qk", 3, 1)],
            (3, 5): [("qk", 7, 0)],
            (3, 6): [("qk", 7, 1)],
        }
        feed_slot = [1]

        def emit_feed(h, s):
            for item in feed.get((h, s), []):
                if item[0] == "v":
                    emit_v_group(item[1], feed_slot[0])
                else:
                    emit_qk_group(item[1], item[2], feed_slot[0])
                feed_slot[0] += 1

        def emit_av(h, s, e, es_e):
            for tj in range(s, NT):
                off = (tj % 4) * 128
                nc.tensor.matmul(
                    u_tiles[e][tj // 4][:, off : off + DV + 1],
                    es_e[:, tj * 128 : (tj + 1) * 128],
                    v_sb[:, s, h, 0 : DV + 1],
                    start=(s == 0 and tj % 4 == 0),
                    stop=(s == tj and tj % 4 == 3),
                )

        def emit_rms(h_lo, h_hi):
            w = (h_hi - h_lo) * NT
            rstd = comb.tile([128, w], F32, tag=f"rstd{h_lo}", name=f"rstd{h_lo}")
            # rstd = (ssq/DV + eps)^-0.5 via ln+exp: stays in the
            # natural_log_exp ACT table set (no table swap mid-exp-stream)
            nc.scalar.activation(
                out=rstd,
                in_=ssq_all[:, h_lo * NT : h_hi * NT],
                func=Act.Ln,
                bias=eps_sb[:],
                scale=1.0 / DV,
            )
            nc.scalar.activation(out=rstd, in_=rstd, func=Act.Exp, scale=-0.5)
            rstd_r = rstd[:].rearrange("p (h t) -> p h t", h=h_hi - h_lo)
            for tj in range(NT):
                nc.vector.tensor_mul(
                    outcat_sb[:, tj, h_lo * DV : h_hi * DV].rearrange(
                        "p (h d) -> p h d", h=h_hi - h_lo
                    ),
                    oh_sb[:, h_lo:h_hi, tj, :],
                    rstd_r[:, :, tj : tj + 1].broadcast_to(
                        [128, h_hi - h_lo, DV]
                    ),
                )

        for h in range(HPG):
            kc_ = 4 + h // 2
            s_pair = s_pool.tile([128, 2, T], F32, tag="sp", name=f"sp{h}")
            es_list = []
            for s in range(NT):
                t0 = 128 * s
                chunks = [(t0, 512), (512, 1024)] if s < 4 else [(t0, 1024)]
                es2 = es_pool.tile([128, 2, T], F16, tag="es", name=f"es_{h}_{s}")
                for c0, c1 in chunks:
                    for e in range(2):
                        nc.tensor.matmul(
                            s_pair[:, e, c0:c1],
                            qkT_sb[:, kc_, t0 : t0 + 128],
                            qTp_sb[:, 2 * h + e, c0:c1],
                            start=True,
                            stop=True,
                        )
                if s == NT - 1 and h < HPG - 1:
                    # stage the last (tiny) S block to SBUF so the next
                    # head's first S matmul waits on a short DVE copy
                    # instead of the exp PSUM read (WAR on s_pair)
                    stg = comb.tile([128, 2, 128], F32, tag="estg", name=f"estg_{h}")
                    nc.vector.tensor_copy(out=stg, in_=s_pair[:, :, t0:T])
                    exp_src = stg[:]
                else:
                    exp_src = s_pair[:, :, t0:T]
                nc.scalar.activation(
                    out=es2[:, :, t0:T],
                    in_=exp_src,
                    func=Act.Exp,
                    scale=1.0 / 32.0,
                )
                nc.gpsimd.affine_select(
                    out=es2[:, :, t0 : t0 + 128],
                    in_=es2[:, :, t0 : t0 + 128],
                    pattern=[[0, 2], [1, 128]],
                    compare_op=AluOp.is_ge,
                    fill=0.0,
                    base=0,
                    channel_multiplier=-1,
                )
                emit_feed(h, s)
                es_list.append(es2)
            # deferred AV block: u banks are only claimed after all of this
            # head's qkv feeds, so the PSUM tag rotation never deadlocks
            u_tiles = [
                [
                    u_pool.tile(
                        [128, 512], F32, tag=mm_tags[2 * e + b], name=f"u{e}{b}_{h}"
                    )
                    for b in range(2)
                ]
                for e in range(2)
            ]
            for s in range(NT):
                emit_av(h, s, 0, es_list[s][:, 0])
                emit_av(h, s, 1, es_list[s][:, 1])

            # batched per-bank epilogue: one reciprocal over 4 denominators,
            # broadcast-AP multiplies over [128, 4, 64], an X-axis reduce for
            # the RMS sum-of-squares.  Strips release in bank order so the
            # next head's AV matmuls start as early as before.
            oh_h = oh_sb[:, h]
            for b in range(2):
                u_r = [
                    u_tiles[e][b][:].rearrange("p (i r) -> p i r", i=4)
                    for e in range(2)
                ]
                rr_ = [
                    comb.tile([128, 4], F32, tag=f"r{e}{b}", name=f"r{e}{b}_{h}")
                    for e in range(2)
                ]
                for e in range(2):
                    nc.vector.reciprocal(out=rr_[e], in_=u_r[e][:, :, DV : DV + 1])
                m1 = comb.tile([128, 4, DV], F32, tag=f"m1{b}", name=f"m1{b}_{h}")
                nc.vector.scalar_tensor_tensor(
                    out=m1,
                    in0=u_r[1][:, :, 0:DV],
                    scalar=lam_sb[:],
                    in1=rr_[1][:, :, None].broadcast_to([128, 4, DV]),
                    op0=AluOp.mult,
                    op1=AluOp.mult,
                )
                m0 = comb.tile([128, 4, DV], F32, tag=f"m0{b}", name=f"m0{b}_{h}")
                nc.vector.tensor_mul(
                    m0,
                    u_r[0][:, :, 0:DV],
                    rr_[0][:, :, None].broadcast_to([128, 4, DV]),
                )
                oh_b = oh_h[:, b * 4 : (b + 1) * 4, :]
                nc.gpsimd.tensor_sub(oh_b, m0, m1)
                # pad the strip stride to DV+1 so the reduce input AP cannot
                # be collapsed to 2D (X-axis reduction keys off the inner dim)
                sq = comb.tile([128, 4, DV + 1], F32, tag=f"sq{b}", name=f"sq{b}_{h}")
                nc.gpsimd.tensor_mul(sq[:, :, 0:DV], oh_b, oh_b)
                nc.vector.tensor_reduce(
                    out=ssq_all[:, h * NT + b * 4 : h * NT + (b + 1) * 4],
                    in_=sq[:, :, 0:DV],
                    axis=mybir.AxisListType.X,
                    op=AluOp.add,
                )
                if h == HPG - 1:
                    # tail: finish head 7 bank-by-bank so the first
                    # transposes (and the output projection behind them)
                    # start while bank 1's epilogue still runs
                    rstd7 = comb.tile([128, 4], F32, tag=f"rstd7{b}", name=f"rstd7{b}")
                    nc.scalar.activation(
                        out=rstd7,
                        in_=ssq_all[:, h * NT + b * 4 : h * NT + (b + 1) * 4],
                        func=Act.Ln,
                        bias=eps_sb[:],
                        scale=1.0 / DV,
                    )
                    nc.scalar.activation(
                        out=rstd7, in_=rstd7, func=Act.Exp, scale=-0.5
                    )
                    for tj in range(b * 4, (b + 1) * 4):
                        nc.vector.tensor_scalar_mul(
                            out=outcat_sb[:, tj, h * DV : (h + 1) * DV],
                            in0=oh_sb[:, h, tj, :],
                            scalar1=rstd7[:, tj - b * 4 : tj - b * 4 + 1],
                        )
                        # split each transpose across two DMA queues: the
                        # transposes gate the output projection tail
                        nc.sync.dma_start_transpose(
                            out=outcatT_sb[:, 0:2, tj * 128 : (tj + 1) * 128],
                            in_=outcat_sb[:, tj, 0:256],
                        )
                        nc.scalar.dma_start_transpose(
                            out=outcatT_sb[:, 2:4, tj * 128 : (tj + 1) * 128],
                            in_=outcat_sb[:, tj, 256:512],
                        )

            if h == 3:
                emit_rms(0, 4)
            elif h == 6:
                emit_rms(4, 7)

    with (
        tc.tile_pool(name="pps", bufs=8, space="PSUM") as pps,
        tc.tile_pool(name="yout", bufs=4) as yout,
    ):
        for tt in range(NT):
            yt = yout.tile([128, C], F16, tag="yt", name=f"y{tt}")
            for nh in range(2):
                ps = pps.tile([128, 512], F32, tag="pp", name=f"pp{tt}{nh}")
                for rr in range(4):
                    nc.tensor.matmul(
                        ps,
                        outcatT_sb[:, rr, tt * 128 : (tt + 1) * 128],
                        wp_sb[:, rr, nh * 512 : (nh + 1) * 512],
                        start=(rr == 0),
                        stop=(rr == 3),
                    )
                nc.vector.tensor_copy(out=yt[:, nh * 512 : (nh + 1) * 512], in_=ps)
            if tt == NT - 1:
                # split the last store across both queues: the final
                # transfer is on the critical tail
                nc.scalar.dma_start(out=y[tt * 128 :, 0:512], in_=yt[:, 0:512])
                nc.sync.dma_start(out=y[tt * 128 :, 512:], in_=yt[:, 512:])
            else:
                eng = nc.scalar if tt % 2 == 0 else nc.sync
                eng.dma_start(out=y[tt * 128 : (tt + 1) * 128, :], in_=yt)


def build_nc():
    nc = bass.Bass()
    prm = {
        "xT": nc.declare_dram_parameter("xT", [C, T], F16, isOutput=False),
        "w_qk": nc.declare_dram_parameter("w_qk", [C, 1024], F16, isOutput=False),
        "w_v": nc.declare_dram_parameter("w_v", [C, 512], F16, isOutput=False),
        "w_p": nc.declare_dram_parameter("w_p", [512, C], F16, isOutput=False),
        "lam": nc.declare_dram_parameter("lam", [128, 1], F32, isOutput=False),
        "zz": nc.declare_dram_parameter("zz", [128, T], F16, isOutput=False),
    }
    y = nc.declare_dram_parameter("y", [T, C], F16, isOutput=True)
    with tile.TileContext(nc) as tc:
        with ExitStack() as ctx:
            _emit(ctx, tc, prm, y)
    return nc


_NC = None


def _get_nc():
    global _NC
    if _NC is None:
        _NC = build_nc()
    return _NC


def make_in_maps(x, w_attn, w_proj, lambda_q1, lambda_q2, lambda_k1, lambda_k2, gamma):
    x = np.asarray(x, np.float32)
    w_attn = np.asarray(w_attn, np.float32)
    w_proj = np.asarray(w_proj, np.float32)
    lam1 = np.exp(np.sum(np.float32(lambda_q1) * np.float32(lambda_k1), dtype=np.float32))
    lam2 = np.exp(np.sum(np.float32(lambda_q2) * np.float32(lambda_k2), dtype=np.float32))
    lam_full = np.float32(lam1 - lam2 + LAMBDA_INIT)
    lam_tile = np.full((128, 1), lam_full, np.float32)
    scale = np.tile(np.asarray(gamma, np.float32), H_TOT) * np.float32(1.0 - LAMBDA_INIT)
    w_p_full = (w_proj * scale[:, None]).astype(np.float16)
    zz = np.zeros((128, T), np.float16)

    in_maps = []
    for core in range(N_CORES):
        b, g = core // G, core % G
        in_maps.append(
            {
                "xT": np.ascontiguousarray(x[b].T.astype(np.float16)),
                "w_qk": np.ascontiguousarray(
                    np.concatenate(
                        [
                            w_attn[:, g * 512 : (g + 1) * 512],
                            w_attn[:, C + g * 512 : C + (g + 1) * 512],
                        ],
                        axis=1,
                    ).astype(np.float16)
                ),
                "w_v": np.ascontiguousarray(
                    w_attn[:, 2 * C + g * 512 : 2 * C + (g + 1) * 512].astype(
                        np.float16
                    )
                ),
                "w_p": np.ascontiguousarray(w_p_full[g * 512 : (g + 1) * 512, :]),
                "lam": lam_tile,
                "zz": zz,
            }
        )
    return in_maps


def assemble(results):
    y = np.empty((B, T, C), np.float32)
    for b in range(B):
        y[b] = results[b * G]["y"].astype(np.float32) + results[b * G + 1][
            "y"
        ].astype(np.float32)
    return y


def kernel(**inputs) -> np.ndarray:
    nc = _get_nc()
    in_maps = make_in_maps(**inputs)
    res = run_bass_kernel_spmd(nc, in_maps, list(range(N_CORES)))
    return assemble(res.results)


# revision 30
# speedup vs baseline: 1.1468x; 1.1468x over previous
"""MultiHeadDiffAttn Trainium2 kernel (v6).

Sharding: 8 cores = 4-way data parallel over batch x 2-way tensor parallel
over heads (8 v-heads / 16 half-heads per core).  Each core computes its
batch's qkv projection restricted to its head group, differential attention
with per-half-head softmax, head RMS norm, and a partial output projection
(its 512 rows of w_proj).  Host sums the two partial projections per batch.

Device-level choices:
  - all matmul operands are fp16 (PSUM accumulation stays fp32); K<128
    matmuls stream at 2 cycles/col, so the S^T matmuls pad the contraction
    to K=128 via a zero-padded per-half-head q buffer (qTp), zeroed in
    per-half-head pieces so the first scatters don't stall on one memset.
  - exp runs once per (half-head, s-block) over the whole PSUM row-block;
    the causal mask is an affine_select on the idle GpSimd engine; the
    s-loop is software-pipelined with AV groups one s-iteration late and
    head 0's first two s-blocks prebaked into the qkv phase.
  - AV accumulates U[t-block, dv|den] directly in PSUM with a ones-column
    as the softmax denominator; normalization / lambda-combine / RMS are
    per-partition DVE ops reading PSUM.
  - rstd = exp(-0.5 * ln(ssq/DV + eps)): both functions live in the single
    natural_log_exp ACT table set, so the mid-stream RMS for heads 0-3
    causes no activation-table swap (the sqrt set would cost ~2.6us twice).
  - y is stored as f16 (the host upcasts and sums the two partials), which
    halves the output DMA on the kernel tail.
"""

import math
from contextlib import ExitStack

import numpy as np

import concourse.bass as bass
import concourse.tile as tile
from concourse import masks, mybir
from concourse.bass_utils import run_bass_kernel_spmd

_MAX_WAITS = 1


def _legalize_sync_waits(d):
    for f in d.get("functions", []):
        for bb in f.get("blocks", []):
            out = []
            for inst in bb["instructions"]:
                si = inst.get("sync_info")
                waits = (si or {}).get("on_wait") or []
                if len(waits) > _MAX_WAITS:
                    extra = waits[: len(waits) - _MAX_WAITS]
                    keep = waits[len(waits) - _MAX_WAITS :]
                    for j in range(0, len(extra), _MAX_WAITS):
                        nop = {
                            "engine": inst["engine"],
                            "ins": [],
                            "outs": [],
                            "name": f"{inst['name']}-lw{j}",
                            "opcode": "NoOp",
                            "sync_info": {
                                "on_wait": extra[j : j + _MAX_WAITS],
                                "on_update": [],
                            },
                        }
                        if "debug" in inst:
                            nop["debug"] = inst["debug"]
                        out.append(nop)
                    si["on_wait"] = keep
                out.append(inst)
            bb["instructions"] = out
    return d


_orig_to_json_bytes = bass.Bass.to_json_bytes


def _patched_to_json_bytes(self, *a, **kw):
    import json as _json

    raw = _orig_to_json_bytes(self, *a, **kw)
    return _json.dumps(_legalize_sync_waits(_json.loads(raw))).encode()


bass.Bass.to_json_bytes = _patched_to_json_bytes

F32 = mybir.dt.float32
F16 = mybir.dt.float16

B, T, C = 4, 1024, 1024
H_TOT = 16
HD = 32
DV = 64
G = 2
HPG = H_TOT // G
COLS = 1024
LAMBDA_INIT = 0.8 - 0.6 * math.exp(-0.3 * (1 - 1))
EPS = 1e-5
N_CORES = 8

NT = T // 128
NKC = C // 128


def _emit(ctx: ExitStack, tc: tile.TileContext, xT, w_qk, w_v, w_p, lam, y):
    nc = tc.nc
    AluOp = mybir.AluOpType
    Act = mybir.ActivationFunctionType

    const = ctx.enter_context(tc.tile_pool(name="const", bufs=1))
    lam_sb = const.tile([128, 1], F32)
    nc.sync.dma_start(out=lam_sb, in_=lam[:])
    eps_sb = const.tile([128, 1], F32)
    nc.vector.memset(eps_sb, EPS)

    big = ctx.enter_context(tc.tile_pool(name="big", bufs=1))
    qkT_sb = big.tile([128, 8, T], F16)
    v_sb = big.tile([128, NT, HPG, 128], F16)
    outcat_sb = big.tile([128, NT, HPG * DV], F16)
    outcatT_sb = big.tile([128, 4, T], F16)
    wp_sb = big.tile([128, 4, C], F16)
    qTp_sb = big.tile([128, 2 * HPG, T], F16)

    es_pool = ctx.enter_context(tc.tile_pool(name="es", bufs=3))
    # zero qTp per half-head so the qk-loop scatters only wait on their own
    # piece instead of one monolithic 13.7us memset
    for hh in range(2 * HPG):
        eng = nc.vector if hh % 2 == 0 else nc.gpsimd
        eng.memset(qTp_sb[:, hh, :], 0.0)

    with (
        tc.tile_pool(name="xw", bufs=1) as xw,
        tc.tile_pool(name="mmps", bufs=4, space="PSUM") as mmps,
    ):
        xT_sb = xw.tile([128, NKC, T], F16)
        wqk_sb = xw.tile([128, NKC, COLS], F16)
        wv_sb = xw.tile([128, NKC, 512], F16)

        xT_r = xT[:].rearrange("(c p) t -> p c t", p=128)
        wqk_r = w_qk[:].rearrange("(c p) m -> p c m", p=128)

        def load_wqk(cc):
            nc.sync.dma_start(
                out=wqk_sb[:, :, cc * 128 : (cc + 1) * 128],
                in_=wqk_r[:, :, cc * 128 : (cc + 1) * 128],
            )

        load_wqk(0)
        for nh in range(2):
            nc.sync.dma_start(
                out=xT_sb[:, :, nh * 512 : (nh + 1) * 512],
                in_=xT_r[:, :, nh * 512 : (nh + 1) * 512],
            )
        for cc in range(1, 8):
            load_wqk(cc)
        nc.sync.dma_start(
            out=wv_sb, in_=w_v[:].rearrange("(c p) m -> p c m", p=128)
        )
        nc.sync.dma_start(
            out=wp_sb, in_=w_p[:].rearrange("(c p) m -> p c m", p=128)
        )

        for cc in range(8):
            for nh in range(2):
                ps = mmps.tile([128, 1024], F32, tag="mmps", name=f"qk{cc}{nh}")[:, 0:512]
                for kc in range(NKC):
                    nc.tensor.matmul(
                        ps,
                        wqk_sb[:, kc, cc * 128 : (cc + 1) * 128],
                        xT_sb[:, kc, nh * 512 : (nh + 1) * 512],
                        start=(kc == 0),
                        stop=(kc == NKC - 1),
                    )
                nc.vector.tensor_copy(
                    out=qkT_sb[:, cc, nh * 512 : (nh + 1) * 512], in_=ps
                )
                if cc < 4:
                    for j in range(4):
                        hh = cc * 4 + j
                        nc.vector.tensor_copy(
                            out=qTp_sb[
                                j * 32 : (j + 1) * 32,
                                hh,
                                nh * 512 : (nh + 1) * 512,
                            ],
                            in_=qkT_sb[
                                j * 32 : (j + 1) * 32,
                                cc,
                                nh * 512 : (nh + 1) * 512,
                            ],
                        )

        prebaked = []
        for s in range(2):
            t0 = 128 * s
            chunks = [(t0, 512), (512, 1024)]
            es2p = es_pool.tile([128, 2, T], F16, tag="es", name=f"esp_{s}")
            for e in range(2):
                sps = mmps.tile([128, 1024], F32, tag="mmps", name=f"sp{e}_{s}")
                for c0, c1 in chunks:
                    nc.tensor.matmul(
                        sps[:, c0:c1],
                        qkT_sb[:, 4, t0 : t0 + 128],
                        qTp_sb[:, e, c0:c1],
                        start=True,
                        stop=True,
                    )
                nc.scalar.activation(
                    out=es2p[:, e, t0:T],
                    in_=sps[:, t0:T],
                    func=Act.Exp,
                    scale=1.0 / 32.0,
                )
            nc.gpsimd.affine_select(
                out=es2p[:, :, t0 : t0 + 128],
                in_=es2p[:, :, t0 : t0 + 128],
                pattern=[[0, 2], [1, 128]],
                compare_op=AluOp.is_ge,
                fill=0.0,
                base=0,
                channel_multiplier=-1,
            )
            prebaked.append((s, es2p, chunks))

        for tt in range(NT):
            ps = mmps.tile([128, 1024], F32, tag="mmps", name=f"v{tt}")[:, 0:512]
            for kc in range(NKC):
                nc.tensor.matmul(
                    ps,
                    xT_sb[:, kc, tt * 128 : (tt + 1) * 128],
                    wv_sb[:, kc, :],
                    start=(kc == 0),
                    stop=(kc == NKC - 1),
                )
            nc.vector.tensor_copy(
                out=v_sb[:, tt, :, 0:DV],
                in_=ps[:].rearrange("p (h d) -> p h d", h=HPG),
            )
            nc.vector.memset(v_sb[:, tt, :, DV : DV + 1], 1.0)

    with (
        tc.tile_pool(name="us", bufs=2) as us_pool,
        tc.tile_pool(name="sps", bufs=1, space="PSUM") as s_pool,
        tc.tile_pool(name="ups", bufs=1, space="PSUM") as u_pool,
        tc.tile_pool(name="comb", bufs=6) as comb,
        tc.tile_pool(name="ohp", bufs=1, space="SBUF") as ohp,
    ):
        oh_sb = ohp.tile([128, HPG, NT, DV], F32, tag="ohall", name="ohall")
        ssq_all = ohp.tile([128, HPG * NT], F32, tag="ssqall", name="ssqall")

        def emit_av(h, s, e, es_s, chunks):
            for tj in range(s, NT):
                off = (tj % 4) * 128
                nc.tensor.matmul(
                    u_tiles[e][tj // 4][:, off : off + DV + 1],
                    es_s[:, tj * 128 : (tj + 1) * 128],
                    v_sb[:, s, h, 0 : DV + 1],
                    start=(s == 0 and tj % 4 == 0),
                    stop=(s == tj and tj % 4 == 3),
                )

        def emit_rms(h_lo, h_hi):
            w = (h_hi - h_lo) * NT
            rstd = comb.tile(
                [128, w], F32, tag=f"rstd{h_lo}", name=f"rstd{h_lo}"
            )
            # rstd = (ssq/DV + eps)^-0.5 via ln+exp: stays in the
            # natural_log_exp ACT table set (no table swap mid-exp-stream)
            nc.scalar.activation(
                out=rstd,
                in_=ssq_all[:, h_lo * NT : h_hi * NT],
                func=Act.Ln,
                bias=eps_sb[:],
                scale=1.0 / DV,
            )
            nc.scalar.activation(out=rstd, in_=rstd, func=Act.Exp, scale=-0.5)
            rstd_r = rstd[:].rearrange("p (h t) -> p h t", h=h_hi - h_lo)
            for tj in range(NT):
                nc.vector.tensor_mul(
                    outcat_sb[:, tj, h_lo * DV : h_hi * DV].rearrange(
                        "p (h d) -> p h d", h=h_hi - h_lo
                    ),
                    oh_sb[:, h_lo:h_hi, tj, :],
                    rstd_r[:, :, tj : tj + 1].broadcast_to(
                        [128, h_hi - h_lo, DV]
                    ),
                )
                if h_hi == HPG:
                    nc.sync.dma_start_transpose(
                        out=outcatT_sb[:, :, tj * 128 : (tj + 1) * 128],
                        in_=outcat_sb[:, tj, :],
                    )

        for h in range(HPG):
            kc_ = 4 + h // 2
            s_pair = s_pool.tile([128, 2, T], F32, tag="sp", name=f"sp{h}")
            u_tiles = [
                [
                    u_pool.tile(
                        [128, 512], F32, tag=f"u{e}{b}", name=f"u{e}{b}_{h}"
                    )
                    for b in range(2)
                ]
                for e in range(2)
            ]
            if h == 0:
                s0_, es0_, ch0_ = prebaked[0]
                emit_av(h, s0_, 0, es0_[:, 0], ch0_)
                emit_av(h, s0_, 1, es0_[:, 1], ch0_)
                prev = prebaked[1]
                s_start = 2
            else:
                prev = None
                s_start = 0
            for s in range(s_start, NT):
                t0 = 128 * s
                chunks = [(t0, 512), (512, 1024)] if s < 4 else [(t0, 1024)]
                es2 = es_pool.tile([128, 2, T], F16, tag="es", name=f"es_{h}_{s}")
                for c0, c1 in chunks:
                    for e in range(2):
                        nc.tensor.matmul(
                            s_pair[:, e, c0:c1],
                            qkT_sb[:, kc_, t0 : t0 + 128],
                            qTp_sb[:, 2 * h + e, c0:c1],
                            start=True,
                            stop=True,
                        )
                if prev is not None:
                    ps_, pes_, pchunks_ = prev
                    emit_av(h, ps_, 0, pes_[:, 0], pchunks_)
                    emit_av(h, ps_, 1, pes_[:, 1], pchunks_)
                if s == NT - 1 and h < HPG - 1:
                    # stage the last (tiny) S block to SBUF so the next
                    # head's first S matmul waits on a short DVE copy
                    # instead of the exp PSUM read (WAR on s_pair)
                    stg = comb.tile(
                        [128, 2, 128], F32, tag="estg", name=f"estg_{h}"
                    )
                    nc.vector.tensor_copy(out=stg, in_=s_pair[:, :, t0:T])
                    exp_src = stg[:]
                else:
                    exp_src = s_pair[:, :, t0:T]
                nc.scalar.activation(
                    out=es2[:, :, t0:T],
                    in_=exp_src,
                    func=Act.Exp,
                    scale=1.0 / 32.0,
                )
                nc.gpsimd.affine_select(
                    out=es2[:, :, t0 : t0 + 128],
                    in_=es2[:, :, t0 : t0 + 128],
                    pattern=[[0, 2], [1, 128]],
                    compare_op=AluOp.is_ge,
                    fill=0.0,
                    base=0,
                    channel_multiplier=-1,
                )
                prev = (s, es2, chunks)
            ps_, pes_, pchunks_ = prev
            emit_av(h, ps_, 0, pes_[:, 0], pchunks_)
            emit_av(h, ps_, 1, pes_[:, 1], pchunks_)

            # batched per-bank epilogue: one reciprocal over 4 denominators,
            # broadcast-AP multiplies over [128, 4, 64], an X-axis reduce for
            # the RMS sum-of-squares.  Strips release in bank order so the
            # next head's AV matmuls start as early as before.
            oh_h = oh_sb[:, h]
            for b in range(2):
                u_r = [
                    u_tiles[e][b][:].rearrange("p (i r) -> p i r", i=4)
                    for e in range(2)
                ]
                rr_ = [
                    comb.tile([128, 4], F32, tag=f"r{e}{b}", name=f"r{e}{b}_{h}")
                    for e in range(2)
                ]
                for e in range(2):
                    nc.vector.reciprocal(out=rr_[e], in_=u_r[e][:, :, DV : DV + 1])
                m1 = comb.tile([128, 4, DV], F32, tag=f"m1{b}", name=f"m1{b}_{h}")
                nc.vector.scalar_tensor_tensor(
                    out=m1,
                    in0=u_r[1][:, :, 0:DV],
                    scalar=lam_sb[:],
                    in1=rr_[1][:, :, None].broadcast_to([128, 4, DV]),
                    op0=AluOp.mult,
                    op1=AluOp.mult,
                )
                m0 = comb.tile([128, 4, DV], F32, tag=f"m0{b}", name=f"m0{b}_{h}")
                nc.vector.tensor_mul(
                    m0,
                    u_r[0][:, :, 0:DV],
                    rr_[0][:, :, None].broadcast_to([128, 4, DV]),
                )
                oh_b = oh_h[:, b * 4 : (b + 1) * 4, :]
                nc.vector.tensor_sub(oh_b, m0, m1)
                # pad the strip stride to DV+1 so the reduce input AP cannot
                # be collapsed to 2D (X-axis reduction keys off the inner dim)
                sq = comb.tile([128, 4, DV + 1], F32, tag=f"sq{b}", name=f"sq{b}_{h}")
                nc.vector.tensor_mul(sq[:, :, 0:DV], oh_b, oh_b)
                nc.vector.tensor_reduce(
                    out=ssq_all[:, h * NT + b * 4 : h * NT + (b + 1) * 4],
                    in_=sq[:, :, 0:DV],
                    axis=mybir.AxisListType.X,
                    op=AluOp.add,
                )
                if h == HPG - 1:
                    # tail: finish head 7 bank-by-bank so the first
                    # transposes (and the output projection behind them)
                    # start while bank 1's epilogue still runs
                    rstd7 = comb.tile(
                        [128, 4], F32, tag=f"rstd7{b}", name=f"rstd7{b}"
                    )
                    nc.scalar.activation(
                        out=rstd7,
                        in_=ssq_all[:, h * NT + b * 4 : h * NT + (b + 1) * 4],
                        func=Act.Ln,
                        bias=eps_sb[:],
                        scale=1.0 / DV,
                    )
                    nc.scalar.activation(
                        out=rstd7, in_=rstd7, func=Act.Exp, scale=-0.5
                    )
                    for tj in range(b * 4, (b + 1) * 4):
                        nc.vector.tensor_scalar_mul(
                            out=outcat_sb[:, tj, h * DV : (h + 1) * DV],
                            in0=oh_sb[:, h, tj, :],
                            scalar1=rstd7[:, tj - b * 4 : tj - b * 4 + 1],
                        )
                        nc.sync.dma_start_transpose(
                            out=outcatT_sb[:, 0:2, tj * 128 : (tj + 1) * 128],
                            in_=outcat_sb[:, tj, 0:256],
                        )
                        nc.scalar.dma_start_transpose(
                            out=outcatT_sb[:, 2:4, tj * 128 : (tj + 1) * 128],
                            in_=outcat_sb[:, tj, 256:512],
                        )

            if h == 3:
                emit_rms(0, 4)
            elif h == 6:
                emit_rms(4, 7)

    with (
        tc.tile_pool(name="tps", bufs=2, space="PSUM") as tps,
        tc.tile_pool(name="pps", bufs=8, space="PSUM") as pps,
        tc.tile_pool(name="yout", bufs=4) as yout,
    ):
        for tt in range(NT):
            yt = yout.tile([128, C], F16, tag="yt", name=f"y{tt}")
            for nh in range(2):
                ps = pps.tile([128, 512], F32, tag="pp", name=f"pp{tt}{nh}")
                for rr in range(4):
                    nc.tensor.matmul(
                        ps,
                        outcatT_sb[:, rr, tt * 128 : (tt + 1) * 128],
                        wp_sb[:, rr, nh * 512 : (nh + 1) * 512],
                        start=(rr == 0),
                        stop=(rr == 3),
                    )
                nc.vector.tensor_copy(out=yt[:, nh * 512 : (nh + 1) * 512], in_=ps)
            if tt == NT - 1:
                # split the last store across both queues: the final
                # transfer is on the critical tail
                nc.scalar.dma_start(out=y[tt * 128 :, 0:512], in_=yt[:, 0:512])
                nc.sync.dma_start(out=y[tt * 128 :, 512:], in_=yt[:, 512:])
            else:
                eng = nc.scalar if tt % 2 == 0 else nc.sync
                eng.dma_start(out=y[tt * 128 : (tt + 1) * 128, :], in_=yt)


def build_nc():
    nc = bass.Bass()
    xT = nc.declare_dram_parameter("xT", [C, T], F16, isOutput=False)
    w_qk = nc.declare_dram_parameter("w_qk", [C, COLS], F16, isOutput=False)
    w_v = nc.declare_dram_parameter("w_v", [C, 512], F16, isOutput=False)
    w_p = nc.declare_dram_parameter("w_p", [512, C], F16, isOutput=False)
    lam = nc.declare_dram_parameter("lam", [128, 1], F32, isOutput=False)
    y = nc.declare_dram_parameter("y", [T, C], F16, isOutput=True)
    with tile.TileContext(nc) as tc:
        with ExitStack() as ctx:
            _emit(ctx, tc, xT, w_qk, w_v, w_p, lam, y)
    return nc


_NC = None


def _get_nc():
    global _NC
    if _NC is None:
        _NC = build_nc()
    return _NC


def make_in_maps(x, w_attn, w_proj, lambda_q1, lambda_q2, lambda_k1, lambda_k2, gamma):
    x = np.asarray(x, np.float32)
    w_attn = np.asarray(w_attn, np.float32)
    w_proj = np.asarray(w_proj, np.float32)
    lam1 = np.exp(np.sum(np.float32(lambda_q1) * np.float32(lambda_k1), dtype=np.float32))
    lam2 = np.exp(np.sum(np.float32(lambda_q2) * np.float32(lambda_k2), dtype=np.float32))
    lam_full = np.float32(lam1 - lam2 + LAMBDA_INIT)
    lam_tile = np.full((128, 1), lam_full, np.float32)
    scale = np.tile(np.asarray(gamma, np.float32), H_TOT) * np.float32(1.0 - LAMBDA_INIT)
    w_p_full = (w_proj * scale[:, None]).astype(np.float16)

    in_maps = []
    for core in range(N_CORES):
        b, g = core // G, core % G
        in_maps.append(
            {
                "xT": np.ascontiguousarray(x[b].T.astype(np.float16)),
                "w_qk": np.ascontiguousarray(
                    np.concatenate(
                        [
                            w_attn[:, g * 512 : (g + 1) * 512],
                            w_attn[:, C + g * 512 : C + (g + 1) * 512],
                        ],
                        axis=1,
                    ).astype(np.float16)
                ),
                "w_v": np.ascontiguousarray(
                    w_attn[:, 2 * C + g * 512 : 2 * C + (g + 1) * 512].astype(
                        np.float16
                    )
                ),
                "w_p": np.ascontiguousarray(w_p_full[g * 512 : (g + 1) * 512, :]),
                "lam": lam_tile,
            }
        )
    return in_maps


def assemble(results):
    y = np.empty((B, T, C), np.float32)
    for b in range(B):
        y[b] = results[b * G]["y"].astype(np.float32) + results[b * G + 1][
            "y"
        ].astype(np.float32)
    return y


def kernel(**inputs) -> np.ndarray:
    nc = _get_nc()
    in_maps = make_in_maps(**inputs)
    res = run_bass_kernel_spmd(nc, in_maps, list(range(N_CORES)))
    return assemble(res.results)



# revision 31
# speedup vs baseline: 1.5620x; 1.3620x over previous
"""MultiHeadDiffAttn Trainium2 kernel (v6).

Sharding: 8 cores = 4-way data parallel over batch x 2-way tensor parallel
over heads (8 v-heads / 16 half-heads per core).  Each core computes its
batch's qkv projection restricted to its head group, differential attention
with per-half-head softmax, head RMS norm, and a partial output projection
(its 512 rows of w_proj).  Host sums the two partial projections per batch.

Device-level choices:
  - all matmul operands are fp16 (PSUM accumulation stays fp32); K<128
    matmuls stream at 2 cycles/col, so the S^T matmuls pad the contraction
    to K=128 via a zero-padded per-half-head q buffer (qTp), zeroed in
    per-half-head pieces so the first scatters don't stall on one memset.
  - exp runs once per (half-head, s-block) over the whole PSUM row-block;
    the causal mask is an affine_select on the idle GpSimd engine; the
    s-loop is software-pipelined with AV groups one s-iteration late and
    head 0's first two s-blocks prebaked into the qkv phase.
  - AV accumulates U[t-block, dv|den] directly in PSUM with a ones-column
    as the softmax denominator; normalization / lambda-combine / RMS are
    per-partition DVE ops reading PSUM.
  - rstd = exp(-0.5 * ln(ssq/DV + eps)): both functions live in the single
    natural_log_exp ACT table set, so the mid-stream RMS for heads 0-3
    causes no activation-table swap (the sqrt set would cost ~2.6us twice).
  - y is stored as f16 (the host upcasts and sums the two partials), which
    halves the output DMA on the kernel tail.
"""

import math
from contextlib import ExitStack

import numpy as np

import concourse.bass as bass
import concourse.tile as tile
from concourse import masks, mybir
from concourse.bass_utils import run_bass_kernel_spmd

_MAX_WAITS = 1


def _legalize_sync_waits(d):
    for f in d.get("functions", []):
        for bb in f.get("blocks", []):
            out = []
            for inst in bb["instructions"]:
                si = inst.get("sync_info")
                waits = (si or {}).get("on_wait") or []
                if len(waits) > _MAX_WAITS:
                    extra = waits[: len(waits) - _MAX_WAITS]
                    keep = waits[len(waits) - _MAX_WAITS :]
                    for j in range(0, len(extra), _MAX_WAITS):
                        nop = {
                            "engine": inst["engine"],
                            "ins": [],
                            "outs": [],
                            "name": f"{inst['name']}-lw{j}",
                            "opcode": "NoOp",
                            "sync_info": {
                                "on_wait": extra[j : j + _MAX_WAITS],
                                "on_update": [],
                            },
                        }
                        if "debug" in inst:
                            nop["debug"] = inst["debug"]
                        out.append(nop)
                    si["on_wait"] = keep
                out.append(inst)
            bb["instructions"] = out
    return d


_orig_to_json_bytes = bass.Bass.to_json_bytes


def _patched_to_json_bytes(self, *a, **kw):
    import json as _json

    raw = _orig_to_json_bytes(self, *a, **kw)
    return _json.dumps(_legalize_sync_waits(_json.loads(raw))).encode()


bass.Bass.to_json_bytes = _patched_to_json_bytes

F32 = mybir.dt.float32
F16 = mybir.dt.float16

B, T, C = 4, 1024, 1024
H_TOT = 16
HD = 32
DV = 64
G = 2
HPG = H_TOT // G
COLS = 1024
LAMBDA_INIT = 0.8 - 0.6 * math.exp(-0.3 * (1 - 1))
EPS = 1e-5
N_CORES = 8

NT = T // 128
NKC = C // 128


def _emit(ctx: ExitStack, tc: tile.TileContext, xT, w_qk, w_v, w_p, lam, y):
    nc = tc.nc
    AluOp = mybir.AluOpType
    Act = mybir.ActivationFunctionType

    const = ctx.enter_context(tc.tile_pool(name="const", bufs=1))
    ident = const.tile([128, 128], F16)
    masks.make_identity(nc, ident[:])
    lam_sb = const.tile([128, 1], F32)
    nc.sync.dma_start(out=lam_sb, in_=lam[:])
    eps_sb = const.tile([128, 1], F32)
    nc.vector.memset(eps_sb, EPS)

    big = ctx.enter_context(tc.tile_pool(name="big", bufs=1))
    qkT_sb = big.tile([128, 8, T], F16)
    v_sb = big.tile([128, NT, HPG, 128], F16)
    outcat_sb = big.tile([128, NT, HPG * DV], F16)
    outcatT_sb = big.tile([128, 4, T], F16)
    wp_sb = big.tile([128, 4, C], F16)
    qTp_sb = big.tile([128, 2 * HPG, T], F16)

    es_pool = ctx.enter_context(tc.tile_pool(name="es", bufs=3))
    # zero qTp per half-head so the qk-loop scatters only wait on their own
    # piece instead of one monolithic 13.7us memset
    for hh in range(2 * HPG):
        eng = nc.vector if hh % 2 == 0 else nc.gpsimd
        eng.memset(qTp_sb[:, hh, :], 0.0)

    with (
        tc.tile_pool(name="xw", bufs=1) as xw,
        tc.tile_pool(name="mmps", bufs=4, space="PSUM") as mmps,
    ):
        xT_sb = xw.tile([128, NKC, T], F16)
        wqk_sb = xw.tile([128, NKC, COLS], F16)
        wv_sb = xw.tile([128, NKC, 512], F16)

        xT_r = xT[:].rearrange("(c p) t -> p c t", p=128)
        wqk_r = w_qk[:].rearrange("(c p) m -> p c m", p=128)

        def load_wqk(cc):
            nc.sync.dma_start(
                out=wqk_sb[:, :, cc * 128 : (cc + 1) * 128],
                in_=wqk_r[:, :, cc * 128 : (cc + 1) * 128],
            )

        load_wqk(0)
        for nh in range(2):
            nc.sync.dma_start(
                out=xT_sb[:, :, nh * 512 : (nh + 1) * 512],
                in_=xT_r[:, :, nh * 512 : (nh + 1) * 512],
            )
        for cc in range(1, 8):
            load_wqk(cc)
        nc.sync.dma_start(
            out=wv_sb, in_=w_v[:].rearrange("(c p) m -> p c m", p=128)
        )
        nc.sync.dma_start(
            out=wp_sb, in_=w_p[:].rearrange("(c p) m -> p c m", p=128)
        )

        for cc in range(8):
            for nh in range(2):
                ps = mmps.tile([128, 1024], F32, tag="mmps", name=f"qk{cc}{nh}")[:, 0:512]
                for kc in range(NKC):
                    nc.tensor.matmul(
                        ps,
                        wqk_sb[:, kc, cc * 128 : (cc + 1) * 128],
                        xT_sb[:, kc, nh * 512 : (nh + 1) * 512],
                        start=(kc == 0),
                        stop=(kc == NKC - 1),
                    )
                nc.vector.tensor_copy(
                    out=qkT_sb[:, cc, nh * 512 : (nh + 1) * 512], in_=ps
                )
                if cc < 4:
                    for j in range(4):
                        hh = cc * 4 + j
                        nc.vector.tensor_copy(
                            out=qTp_sb[
                                j * 32 : (j + 1) * 32,
                                hh,
                                nh * 512 : (nh + 1) * 512,
                            ],
                            in_=qkT_sb[
                                j * 32 : (j + 1) * 32,
                                cc,
                                nh * 512 : (nh + 1) * 512,
                            ],
                        )

        prebaked = []
        for s in range(2):
            t0 = 128 * s
            chunks = [(t0, 512), (512, 1024)]
            es_s = [
                es_pool.tile([128, T], F16, tag=f"es{e}", name=f"esp{e}_{s}")
                for e in range(2)
            ]
            for e in range(2):
                sps = mmps.tile([128, 1024], F32, tag="mmps", name=f"sp{e}_{s}")
                for c0, c1 in chunks:
                    nc.tensor.matmul(
                        sps[:, c0:c1],
                        qkT_sb[:, 4, t0 : t0 + 128],
                        qTp_sb[:, e, c0:c1],
                        start=True,
                        stop=True,
                    )
                nc.scalar.activation(
                    out=es_s[e][:, t0:T],
                    in_=sps[:, t0:T],
                    func=Act.Exp,
                    scale=1.0 / 32.0,
                )
                nc.gpsimd.affine_select(
                    out=es_s[e][:, t0 : t0 + 128],
                    in_=es_s[e][:, t0 : t0 + 128],
                    pattern=[[1, 128]],
                    compare_op=AluOp.is_ge,
                    fill=0.0,
                    base=0,
                    channel_multiplier=-1,
                )
            prebaked.append((s, es_s, chunks))

        for tt in range(NT):
            ps = mmps.tile([128, 1024], F32, tag="mmps", name=f"v{tt}")[:, 0:512]
            for kc in range(NKC):
                nc.tensor.matmul(
                    ps,
                    xT_sb[:, kc, tt * 128 : (tt + 1) * 128],
                    wv_sb[:, kc, :],
                    start=(kc == 0),
                    stop=(kc == NKC - 1),
                )
            nc.vector.tensor_copy(
                out=v_sb[:, tt, :, 0:DV],
                in_=ps[:].rearrange("p (h d) -> p h d", h=HPG),
            )
            nc.vector.memset(v_sb[:, tt, :, DV : DV + 1], 1.0)

    with (
        tc.tile_pool(name="us", bufs=2) as us_pool,
        tc.tile_pool(name="sps", bufs=1, space="PSUM") as s_pool,
        tc.tile_pool(name="ups", bufs=1, space="PSUM") as u_pool,
        tc.tile_pool(name="comb", bufs=6) as comb,
        tc.tile_pool(name="ohp", bufs=1, space="SBUF") as ohp,
    ):
        oh_sb = ohp.tile([128, HPG, NT, DV], F32, tag="ohall", name="ohall")
        ssq_all = ohp.tile([128, HPG * NT], F32, tag="ssqall", name="ssqall")

        def emit_av(h, s, e, es_s, chunks):
            for tj in range(s, NT):
                off = (tj % 4) * 128
                nc.tensor.matmul(
                    u_tiles[e][tj // 4][:, off : off + DV + 1],
                    es_s[:, tj * 128 : (tj + 1) * 128],
                    v_sb[:, s, h, 0 : DV + 1],
                    start=(s == 0 and tj % 4 == 0),
                    stop=(s == tj and tj % 4 == 3),
                )

        def emit_rms(h_lo, h_hi):
            w = (h_hi - h_lo) * NT
            rstd = comb.tile(
                [128, w], F32, tag=f"rstd{h_lo}", name=f"rstd{h_lo}"
            )
            # rstd = (ssq/DV + eps)^-0.5 via ln+exp: stays in the
            # natural_log_exp ACT table set (no table swap mid-exp-stream)
            nc.scalar.activation(
                out=rstd,
                in_=ssq_all[:, h_lo * NT : h_hi * NT],
                func=Act.Ln,
                bias=eps_sb[:],
                scale=1.0 / DV,
            )
            nc.scalar.activation(out=rstd, in_=rstd, func=Act.Exp, scale=-0.5)
            rstd_r = rstd[:].rearrange("p (h t) -> p h t", h=h_hi - h_lo)
            for tj in range(NT):
                nc.vector.tensor_mul(
                    outcat_sb[:, tj, h_lo * DV : h_hi * DV].rearrange(
                        "p (h d) -> p h d", h=h_hi - h_lo
                    ),
                    oh_sb[:, h_lo:h_hi, tj, :],
                    rstd_r[:, :, tj : tj + 1].broadcast_to(
                        [128, h_hi - h_lo, DV]
                    ),
                )
                if h_hi == HPG:
                    nc.sync.dma_start_transpose(
                        out=outcatT_sb[:, :, tj * 128 : (tj + 1) * 128],
                        in_=outcat_sb[:, tj, :],
                    )

        for h in range(HPG):
            qc = h // 2
            kc_ = 4 + h // 2
            pbase = [(2 * h % 4) * 32, (2 * h % 4) * 32 + 32]
            s_tiles = [
                s_pool.tile([128, T], F32, tag=f"s{e}", name=f"s{e}_{h}")
                for e in range(2)
            ]
            u_tiles = [
                [
                    u_pool.tile(
                        [128, 512], F32, tag=f"u{e}{b}", name=f"u{e}{b}_{h}"
                    )
                    for b in range(2)
                ]
                for e in range(2)
            ]
            if h == 0:
                s0_, es0_, ch0_ = prebaked[0]
                emit_av(h, s0_, 0, es0_[0], ch0_)
                emit_av(h, s0_, 1, es0_[1], ch0_)
                prev = prebaked[1]
                s_start = 2
            else:
                prev = None
                s_start = 0
            for s in range(s_start, NT):
                t0 = 128 * s
                chunks = [(t0, 512), (512, 1024)] if s < 4 else [(t0, 1024)]
                es_s = [
                    es_pool.tile([128, T], F16, tag=f"es{e}", name=f"es{e}_{h}_{s}")
                    for e in range(2)
                ]
                for c0, c1 in chunks:
                    for e in range(2):
                        nc.tensor.matmul(
                            s_tiles[e][:, c0:c1],
                            qkT_sb[:, kc_, t0 : t0 + 128],
                            qTp_sb[:, 2 * h + e, c0:c1],
                            start=True,
                            stop=True,
                        )
                if prev is not None:
                    ps_, pes_, pchunks_ = prev
                    emit_av(h, ps_, 0, pes_[0], pchunks_)
                    emit_av(h, ps_, 1, pes_[1], pchunks_)
                if s == NT - 1 and h < HPG - 1:
                    # stage the last (tiny) S block to SBUF so the next
                    # head's first S matmul waits on a ~150ns DVE copy
                    # instead of the ~700ns exp PSUM read (WAR on s_tiles)
                    stg = comb.tile(
                        [128, 2, 128], F32, tag="estg", name=f"estg_{h}"
                    )
                    for e in range(2):
                        nc.vector.tensor_copy(
                            out=stg[:, e], in_=s_tiles[e][:, t0:T]
                        )
                    exp_srcs = [stg[:, 0], stg[:, 1]]
                else:
                    exp_srcs = [s_tiles[e][:, t0:T] for e in range(2)]
                for e in range(2):
                    nc.scalar.activation(
                        out=es_s[e][:, t0:T],
                        in_=exp_srcs[e],
                        func=Act.Exp,
                        scale=1.0 / 32.0,
                    )
                    nc.gpsimd.affine_select(
                        out=es_s[e][:, t0 : t0 + 128],
                        in_=es_s[e][:, t0 : t0 + 128],
                        pattern=[[1, 128]],
                        compare_op=AluOp.is_ge,
                        fill=0.0,
                        base=0,
                        channel_multiplier=-1,
                    )
                prev = (s, es_s, chunks)
            ps_, pes_, pchunks_ = prev
            emit_av(h, ps_, 0, pes_[0], pchunks_)
            emit_av(h, ps_, 1, pes_[1], pchunks_)

            # batched per-bank epilogue: one reciprocal over 4 denominators,
            # broadcast-AP multiplies over [128, 4, 64], an X-axis reduce for
            # the RMS sum-of-squares.  Strips release in bank order so the
            # next head's AV matmuls start as early as before.
            oh_h = oh_sb[:, h]
            for b in range(2):
                u_r = [
                    u_tiles[e][b][:].rearrange("p (i r) -> p i r", i=4)
                    for e in range(2)
                ]
                rr_ = [
                    comb.tile([128, 4], F32, tag=f"r{e}{b}", name=f"r{e}{b}_{h}")
                    for e in range(2)
                ]
                for e in range(2):
                    nc.vector.reciprocal(out=rr_[e], in_=u_r[e][:, :, DV : DV + 1])
                m1 = comb.tile([128, 4, DV], F32, tag=f"m1{b}", name=f"m1{b}_{h}")
                nc.vector.scalar_tensor_tensor(
                    out=m1,
                    in0=u_r[1][:, :, 0:DV],
                    scalar=lam_sb[:],
                    in1=rr_[1][:, :, None].broadcast_to([128, 4, DV]),
                    op0=AluOp.mult,
                    op1=AluOp.mult,
                )
                m0 = comb.tile([128, 4, DV], F32, tag=f"m0{b}", name=f"m0{b}_{h}")
                nc.vector.tensor_mul(
                    m0,
                    u_r[0][:, :, 0:DV],
                    rr_[0][:, :, None].broadcast_to([128, 4, DV]),
                )
                oh_b = oh_h[:, b * 4 : (b + 1) * 4, :]
                nc.vector.tensor_sub(oh_b, m0, m1)
                # pad the strip stride to DV+1 so the reduce input AP cannot
                # be collapsed to 2D (X-axis reduction keys off the inner dim)
                sq = comb.tile([128, 4, DV + 1], F32, tag=f"sq{b}", name=f"sq{b}_{h}")
                nc.vector.tensor_mul(sq[:, :, 0:DV], oh_b, oh_b)
                nc.vector.tensor_reduce(
                    out=ssq_all[:, h * NT + b * 4 : h * NT + (b + 1) * 4],
                    in_=sq[:, :, 0:DV],
                    axis=mybir.AxisListType.X,
                    op=AluOp.add,
                )
                if h == HPG - 1:
                    # tail: finish head 7 bank-by-bank so the first
                    # transposes (and the output projection behind them)
                    # start while bank 1's epilogue still runs
                    rstd7 = comb.tile(
                        [128, 4], F32, tag=f"rstd7{b}", name=f"rstd7{b}"
                    )
                    nc.scalar.activation(
                        out=rstd7,
                        in_=ssq_all[:, h * NT + b * 4 : h * NT + (b + 1) * 4],
                        func=Act.Ln,
                        bias=eps_sb[:],
                        scale=1.0 / DV,
                    )
                    nc.scalar.activation(
                        out=rstd7, in_=rstd7, func=Act.Exp, scale=-0.5
                    )
                    for tj in range(b * 4, (b + 1) * 4):
                        nc.vector.tensor_scalar_mul(
                            out=outcat_sb[:, tj, h * DV : (h + 1) * DV],
                            in0=oh_sb[:, h, tj, :],
                            scalar1=rstd7[:, tj - b * 4 : tj - b * 4 + 1],
                        )
                        teng = nc.sync if b == 0 else nc.scalar
                        teng.dma_start_transpose(
                            out=outcatT_sb[:, :, tj * 128 : (tj + 1) * 128],
                            in_=outcat_sb[:, tj, :],
                        )

            if h == 3:
                emit_rms(0, 4)
            elif h == 6:
                emit_rms(4, 7)

    with (
        tc.tile_pool(name="tps", bufs=2, space="PSUM") as tps,
        tc.tile_pool(name="pps", bufs=8, space="PSUM") as pps,
        tc.tile_pool(name="yout", bufs=4) as yout,
    ):
        for tt in range(NT):
            yt = yout.tile([128, C], F16, tag="yt", name=f"y{tt}")
            for nh in range(2):
                ps = pps.tile([128, 512], F32, tag="pp", name=f"pp{tt}{nh}")
                for rr in range(4):
                    nc.tensor.matmul(
                        ps,
                        outcatT_sb[:, rr, tt * 128 : (tt + 1) * 128],
                        wp_sb[:, rr, nh * 512 : (nh + 1) * 512],
                        start=(rr == 0),
                        stop=(rr == 3),
                    )
                nc.vector.tensor_copy(out=yt[:, nh * 512 : (nh + 1) * 512], in_=ps)
            if tt == NT - 1:
                # split the last store across both queues: the final
                # transfer is on the critical tail
                nc.scalar.dma_start(out=y[tt * 128 :, 0:512], in_=yt[:, 0:512])
                nc.sync.dma_start(out=y[tt * 128 :, 512:], in_=yt[:, 512:])
            else:
                eng = nc.scalar if tt % 2 == 0 else nc.sync
                eng.dma_start(out=y[tt * 128 : (tt + 1) * 128, :], in_=yt)


def build_nc():
    nc = bass.Bass()
    xT = nc.declare_dram_parameter("xT", [C, T], F16, isOutput=False)
    w_qk = nc.declare_dram_parameter("w_qk", [C, COLS], F16, isOutput=False)
    w_v = nc.declare_dram_parameter("w_v", [C, 512], F16, isOutput=False)
    w_p = nc.declare_dram_parameter("w_p", [512, C], F16, isOutput=False)
    lam = nc.declare_dram_parameter("lam", [128, 1], F32, isOutput=False)
    y = nc.declare_dram_parameter("y", [T, C], F16, isOutput=True)
    with tile.TileContext(nc) as tc:
        with ExitStack() as ctx:
            _emit(ctx, tc, xT, w_qk, w_v, w_p, lam, y)
    return nc


_NC = None


def _get_nc():
    global _NC
    if _NC is None:
        _NC = build_nc()
    return _NC


def make_in_maps(x, w_attn, w_proj, lambda_q1, lambda_q2, lambda_k1, lambda_k2, gamma):
    x = np.asarray(x, np.float32)
    w_attn = np.asarray(w_attn, np.float32)
    w_proj = np.asarray(w_proj, np.float32)
    lam1 = np.exp(np.sum(np.float32(lambda_q1) * np.float32(lambda_k1), dtype=np.float32))
    lam2 = np.exp(np.sum(np.float32(lambda_q2) * np.float32(lambda_k2), dtype=np.float32))
    lam_full = np.float32(lam1 - lam2 + LAMBDA_INIT)
    lam_tile = np.full((128, 1), lam_full, np.float32)
    scale = np.tile(np.asarray(gamma, np.float32), H_TOT) * np.float32(1.0 - LAMBDA_INIT)
    w_p_full = (w_proj * scale[:, None]).astype(np.float16)

    in_maps = []
    for core in range(N_CORES):
        b, g = core // G, core % G
        in_maps.append(
            {
                "xT": np.ascontiguousarray(x[b].T.astype(np.float16)),
                "w_qk": np.ascontiguousarray(
                    np.concatenate(
                        [
                            w_attn[:, g * 512 : (g + 1) * 512],
                            w_attn[:, C + g * 512 : C + (g + 1) * 512],
                        ],
                        axis=1,
                    ).astype(np.float16)
                ),
                "w_v": np.ascontiguousarray(
                    w_attn[:, 2 * C + g * 512 : 2 * C + (g + 1) * 512].astype(
                        np.float16
                    )
                ),
                "w_p": np.ascontiguousarray(w_p_full[g * 512 : (g + 1) * 512, :]),
                "lam": lam_tile,
            }
        )
    return in_maps


def assemble(results):
    y = np.empty((B, T, C), np.float32)
    for b in range(B):
        y[b] = results[b * G]["y"].astype(np.float32) + results[b * G + 1][
            "y"
        ].astype(np.float32)
    return y


def kernel(**inputs) -> np.ndarray:
    nc = _get_nc()
    in_maps = make_in_maps(**inputs)
    res = run_bass_kernel_spmd(nc, in_maps, list(range(N_CORES)))
    return assemble(res.results)

